# revision 51
# baseline (speedup 1.0000x reference)
"""Multi-head attention (B=8, N=1024, C=1024, H=16) on 8 Trainium2 NeuronCores.

Sharding: pure data-parallel — one batch element per core, weights replicated,
no collectives.

v2 design (vs baseline): bf16 matmul operands everywhere (PSUM accumulation
stays fp32), no DRAM bounce for qk (everything lives in SBUF), per-head-pair
software pipeline so the PE and ACT engines run concurrently, 1024-wide exp
activations (psum tiles spanning 2 banks) to amortize ACT fixed overhead, and
row-packed S matmuls (K=64 head A on array rows 0-63, head B on rows 64-127,
issued back-to-back so they execute concurrently).

Per-core algorithm:
  v-proj:    v[m, dv] natural layout, interleaved [m, 16*(64+1)] with a ones
             column per head (PV then emits softmax denominators for free).
  qk-proj:   per pair p: qp[c(2 heads), n], kp[c, m] bf16 tiles in SBUF.
  attention: per (pair, nt half):
               4 groups: S^T chunks for heads A,B into [128,1024] psum pairs,
               exp (ACT, 1024-wide, scale=1/8) -> eA/eB bf16 [128, 4096]
               PV: U_aug[65, nt] = v_aug.T @ expS accumulated over 8 m-chunks
             U -> SBUF (fp32, frees psum), denominators row 64 -> reciprocal
             -> DRAM bounce -> partition-broadcast -> normalize on GpSimd into
             attn_outT[c, n] bf16.
  out-proj:  out[n, d] = attn_outT.T @ wpT + bias, fp32 out.
"""

import sys

if "/opt/trn_rl_repo" not in sys.path:
    sys.path.insert(0, "/opt/trn_rl_repo")

from contextlib import ExitStack

import numpy as np

import concourse.bass as bass
import concourse.mybir as mybir
from concourse import bacc
import concourse.tile as tile
from concourse import bass_utils

B, N, C, H = 8, 1024, 1024, 16
HD = C // H          # 64
SCALE = HD ** -0.5   # 0.125
P = 128              # SBUF partitions
NT = 512             # moving-dim tile (fp32 PSUM bank limit)
NCH = C // P         # 8 contraction chunks over channels
NMT = N // P         # 8 token tiles of 128
NNT = N // NT        # 2 token tiles of 512
NPAIR = H // 2       # 8 head pairs
F32 = mybir.dt.float32
BF16 = mybir.dt.bfloat16
EXP = mybir.ActivationFunctionType.Exp


def _wait_key(w):
    return (w.sync_type, w.id, w.wait_mode, w.wait_value)


def _weights_sig(ldw):
    a = ldw.ins[0]
    return (a.memref, a.offset, tuple(tuple(x) for x in a.ap), str(a.dtype))


def _optimize_pe_stream(nc):
    """Post-compile peephole pass over the PE instruction stream.

    Operates on the PE-only subsequence (other engines' instructions are
    interleaved in the block list but the PE sequencer only sees its own
    stream; cross-engine ordering is carried entirely by semaphores).

    Rule 1 (dedupe): a wait-free LDWEIGHTS reloading the weights already in
      the array (and already consumed by a matmul) is deleted. Wait-carrying
      LDWs are kept: waits only function on LDWEIGHTS (the PE hw-decoder
      ignores waits on MATMUL), and the first load of a compiler
      [LDW, LDW, MM, MM] prefetch pair may run before the DMA-complete wait.
    Rule 2 (hoist): [LDW_A, MM_A@(0,0) K=64, LDW_B, MM_B@(64,0) K=64] ->
      [LDW_A, LDW_B, MM_A, MM_B] so the two matmuls execute concurrently on
      disjoint row groups; only when LDW_B's waits are implied by LDW_A's
      (same semaphore, same-or-lower threshold), so the earlier wait position
      cannot deadlock.
    """
    import concourse.mybir as mybir
    from collections import deque

    n_dedupe = n_hoist = 0
    for f in nc.m.functions:
        for blk in f.blocks:
            insts = blk.instructions
            pe = [a for a in insts if getattr(a, "engine", None) == mybir.EngineType.PE]
            out = []
            i = 0
            n = len(pe)
            cur_sig = None
            cur_consumed = False
            while i < n:
                a = pe[i]
                if isinstance(a, mybir.InstLdweights):
                    sig = _weights_sig(a)
                    if (
                        sig == cur_sig
                        and cur_consumed
                        and (
                            a.sync_info is None
                            or not (a.sync_info.on_wait or a.sync_info.on_update)
                        )
                        and i + 1 < n
                        and isinstance(pe[i + 1], mybir.InstMatmult)
                    ):
                        out.append(pe[i + 1])
                        i += 2
                        n_dedupe += 1
                        cur_consumed = True
                        continue
                    if (
                        i + 3 < n
                        and isinstance(pe[i + 1], mybir.InstMatmult)
                        and isinstance(pe[i + 2], mybir.InstLdweights)
                        and isinstance(pe[i + 3], mybir.InstMatmult)
                    ):
                        ldw1, mm1, ldw2, mm2 = pe[i : i + 4]
                        tp1 = mm1.tile_position
                        tp2 = mm2.tile_position
                        if (
                            tp1 is not None
                            and tp2 is not None
                            and tuple(tp1) == (0, 0)
                            and tuple(tp2) == (64, 0)
                            and ldw1.ins[0].ap[0][1] == 64
                            and ldw2.ins[0].ap[0][1] == 64
                        ):
                            w1 = [
                                _wait_key(w)
                                for w in (
                                    ldw1.sync_info.on_wait if ldw1.sync_info else []
                                )
                            ]
                            w2 = [
                                _wait_key(w)
                                for w in (
                                    ldw2.sync_info.on_wait if ldw2.sync_info else []
                                )
                            ]
                            implied = all(
                                any(
                                    k[0] == kk[0]
                                    and k[1] == kk[1]
                                    and k[2] == kk[2]
                                    and k[3] <= kk[3]
                                    for kk in w1
                                )
                                for k in w2
                            )
                            if implied and not (
                                ldw2.sync_info and ldw2.sync_info.on_update
                            ):
                                out.extend([ldw1, ldw2, mm1, mm2])
                                cur_sig = _weights_sig(ldw2)
                                cur_consumed = True
                                i += 4
                                n_hoist += 1
                                continue
                    cur_sig = sig
                    cur_consumed = False
                elif isinstance(a, mybir.InstMatmult):
                    cur_consumed = True
                else:
                    cur_sig = None  # unknown PE instruction: be conservative
                    cur_consumed = False
                out.append(a)
                i += 1
            # weave the transformed PE stream back into the block, keeping
            # non-PE instructions in place; trailing PE slots left over from
            # deletions are simply skipped.
            pe_q = deque(out)
            new_insts = []
            for a in insts:
                if getattr(a, "engine", None) == mybir.EngineType.PE:
                    if pe_q:
                        new_insts.append(pe_q.popleft())
                else:
                    new_insts.append(a)
            assert not pe_q, "transformed PE stream longer than original slots"
            blk.instructions = new_insts
    print(f"_optimize_pe_stream: {n_dedupe} LDW deduped, {n_hoist} LDW hoisted")


def build_module():
    nc = bacc.Bacc("TRN2", target_bir_lowering=False, debug=False, num_devices=B)

    xT = nc.dram_tensor("xT", [C, N], BF16, kind="ExternalInput").ap()
    wqkT = nc.dram_tensor("wqkT", [C, 2 * C], BF16, kind="ExternalInput").ap()
    wvT = nc.dram_tensor("wvT", [C, C], BF16, kind="ExternalInput").ap()
    wpT = nc.dram_tensor("wpT", [C, C], BF16, kind="ExternalInput").ap()
    bias = nc.dram_tensor("bias_bc", [P, C], F32, kind="ExternalInput").ap()
    ones_col = nc.dram_tensor("ones_col", [P, H], BF16, kind="ExternalInput").ap()
    out = nc.dram_tensor("out", [N, C], F32, kind="ExternalOutput").ap()

    with tile.TileContext(nc) as tc, ExitStack() as ctx:
        dram = ctx.enter_context(tc.tile_pool(name="dram", bufs=1, space="DRAM"))
        rden_d = dram.tile([H * NNT, NT], F32, tag="rden_d", name="rden_d")

        xt_pool = ctx.enter_context(tc.tile_pool(name="xt", bufs=8))
        qk_pool = ctx.enter_context(tc.tile_pool(name="qk", bufs=8))
        v_pool = ctx.enter_context(tc.tile_pool(name="v", bufs=8))
        e_pool = ctx.enter_context(tc.tile_pool(name="e", bufs=8))
        u_pool = ctx.enter_context(tc.tile_pool(name="u", bufs=6))
        aot_pool = ctx.enter_context(tc.tile_pool(name="aot", bufs=1))
        w_pool = ctx.enter_context(tc.tile_pool(name="wst", bufs=9))
        wqk_pool = ctx.enter_context(tc.tile_pool(name="wqk", bufs=24))
        den_pool = ctx.enter_context(tc.tile_pool(name="den", bufs=4))
        rbc_pool = ctx.enter_context(tc.tile_pool(name="rbc", bufs=6))
        one_pool = ctx.enter_context(tc.tile_pool(name="one", bufs=1))
        stage_pool = ctx.enter_context(tc.tile_pool(name="stage", bufs=2))
        s_psum = ctx.enter_context(tc.tile_pool(name="s_ps", bufs=2, space="PSUM"))
        pv_psum = ctx.enter_context(tc.tile_pool(name="pv_ps", bufs=2, space="PSUM"))
        pj_psum = ctx.enter_context(tc.tile_pool(name="pj_ps", bufs=1, space="PSUM"))

        # ---------- input loads ----------
        xts = []
        for t in range(NCH):
            xt_t = xt_pool.tile([P, N], BF16, tag="xt", name=f"xt{t}")
            nc.sync.dma_start(xt_t, xT[t * P : (t + 1) * P, :])
            xts.append(xt_t)
        vsb = []
        for mt in range(NMT):
            v_t = v_pool.tile([P, H * (HD + 1)], BF16, tag="v", name=f"v{mt}")
            nc.sync.dma_start(
                v_t.rearrange("p (h w) -> p h w", w=HD + 1)[:, :, HD : HD + 1], ones_col
            )
            vsb.append(v_t)
        aot = [
            aot_pool.tile([P, N], BF16, tag=f"aot{t}", name=f"aot{t}")
            for t in range(NCH)
        ]

        # ---------- v projection (natural layout + ones cols) ----------
        # Runs on the pv psum banks (idle until the first PV at ~50us) so the
        # projection chain proceeds concurrently with the qk-proj chain on the
        # pj slot; per-dvt [128,512] blocks ping-pong the two pv banks. PV(p)
        # only needs vsb[mc], so it chases these blocks per-mt.
        def emit_vproj():
            wv_tiles = []
            for ck in range(NCH):
                wv_t = w_pool.tile([P, N], BF16, tag="wst", name=f"wv{ck}")
                nc.sync.dma_start(wv_t, wvT[ck * P : (ck + 1) * P, :])
                wv_tiles.append(wv_t)
            for dvt in range(NNT):
                for mt in range(NMT):
                    ps = pv_psum.tile([P, NT], F32, tag="pv", name=f"psv{dvt}_{mt}")
                    for ck in range(NCH):
                        nc.tensor.matmul(
                            ps,
                            lhsT=xts[ck][:, mt * P : (mt + 1) * P],
                            rhs=wv_tiles[ck][:, dvt * NT : (dvt + 1) * NT],
                            start=(ck == 0),
                            stop=(ck == NCH - 1),
                        )
                    dst = vsb[mt].rearrange("p (h w) -> p h w", w=HD + 1)[
                        :, dvt * 8 : (dvt + 1) * 8, 0:HD
                    ]
                    nc.vector.tensor_copy(dst, ps.rearrange("p (h w) -> p h w", w=HD))

        # ---------- qk projection for one head pair, SBUF-resident ----------
        def emit_qkproj(p):
            qp = qk_pool.tile([P, N], BF16, tag="qk", name=f"qp{p}")
            kp = qk_pool.tile([P, N], BF16, tag="qk", name=f"kp{p}")
            for which, col0, dstt in ((0, p * P, qp), (1, C + p * P, kp)):
                wts = []
                for ck in range(NCH):
                    w_t = wqk_pool.tile(
                        [P, P], BF16, tag="wqk", name=f"w{which}_{p}_{ck}"
                    )
                    nc.sync.dma_start(w_t, wqkT[ck * P : (ck + 1) * P, col0 : col0 + P])
                    wts.append(w_t)
                ps = pj_psum.tile([P, 2 * NT], F32, tag="pj", name=f"psqk{which}_{p}")
                for ck in range(NCH):
                    for nt_ in range(NNT):
                        nc.tensor.matmul(
                            ps[:, nt_ * NT : (nt_ + 1) * NT],
                            lhsT=wts[ck],
                            rhs=xts[ck][:, nt_ * NT : (nt_ + 1) * NT],
                            start=(ck == 0),
                            stop=(ck == NCH - 1),
                        )
                nc.vector.tensor_copy(dstt, ps)
            return qp, kp

        # ---------- attention ----------
        pair_units = {}

        def emit_denorm(p, nt_, punits):
            # Per n-half (2 units, not 4): shortens the tail chain — out-proj
            # blocks 0-3 only read the nt0 half of aot, so the last pair's
            # first denominators unblock them earlier.
            den_g = den_pool.tile([2, NT], F32, tag="den", name=f"den{p}_{nt_}")
            for i, (h, u_t) in enumerate(punits):
                nc.sync.dma_start(den_g[i : i + 1, :], u_t[HD : HD + 1, :])
            rden = den_pool.tile([2, NT], F32, tag="rden", name=f"rden{p}_{nt_}")
            nc.vector.reciprocal_approx_fast(out=rden, in_=den_g)
            row0 = p * 4 + nt_ * 2
            nc.sync.dma_start(rden_d[row0 : row0 + 2, :], rden)
            for i, (h, u_t) in enumerate(punits):
                rbc = rbc_pool.tile([HD, NT], F32, tag="rbc", name=f"rbc{h}_{nt_}")
                src_ = rden_d[row0 + i : row0 + i + 1, :]
                bsrc = bass.AP(
                    tensor=src_.tensor,
                    offset=src_.offset,
                    ap=[[0, HD], list(src_.ap[-1])],
                )
                nc.sync.dma_start(out=rbc, in_=bsrc)
                ct, prow = h // 2, (h % 2) * HD
                # split normalize across GpSimd and Vector so the two heads'
                # muls run on different engines (matters for the last pair's
                # tail, which gates out-proj).
                eng = nc.gpsimd if h % 2 == 0 else nc.vector
                eng.tensor_mul(
                    aot[ct][prow : prow + HD, nt_ * NT : (nt_ + 1) * NT],
                    u_t[0:HD, :],
                    rbc,
                )

        def emit_s_exp_nt(p, nt_, qp, kp):
            """S^T + exp for one (pair, n-half). Heads A and B share one
            [128, 1024] psum tile per m-chunk (A in the low bank, B in the
            high bank) so both matmuls become ready together; the post-compile
            pass hoists B's LDWEIGHTS above A's matmul, making the two K=64
            matmuls (array rows 0-63 / 64-127) run concurrently.
            Returns two e tiles [128, 4096] (mc 0-3 and mc 4-7), each laid
            out as [A_mc|B_mc|...]; the split lets PV release the first half
            mid-chain so the next pair's S can reuse the slots earlier."""
            eA = e_pool.tile([P, NMT * NT], BF16, tag="e", name=f"e{p}_{nt_}a")
            eB = e_pool.tile([P, NMT * NT], BF16, tag="e", name=f"e{p}_{nt_}b")
            for mc in range(NMT):
                e_t = eA if mc < 4 else eB
                s_t = s_psum.tile([P, 2 * NT], F32, tag="s", name=f"s{p}_{nt_}_{mc}")
                # high priority: the S pair feeds ACT (the attention-phase
                # pacer) and must pop back-to-back so the post-compile hoist
                # can make the two K=64 row-tiles run concurrently.
                with tc.high_priority():
                    nc.tensor.matmul(
                        s_t[:, 0:NT],
                        lhsT=kp[0:HD, mc * P : (mc + 1) * P],
                        rhs=qp[0:HD, nt_ * NT : (nt_ + 1) * NT],
                        start=True,
                        stop=True,
                    )
                    nc.tensor.matmul(
                        s_t[:, NT : 2 * NT],
                        lhsT=kp[HD:P, mc * P : (mc + 1) * P],
                        rhs=qp[HD:P, nt_ * NT : (nt_ + 1) * NT],
                        start=True,
                        stop=True,
                    )
                nc.scalar.activation(
                    e_t[:, (mc % 4) * 2 * NT : ((mc % 4) + 1) * 2 * NT],
                    s_t,
                    EXP,
                    scale=SCALE,
                )
            return eA, eB

        def emit_pv_nt(p, nt_, e_h):
            """PV for BOTH heads of the pair over one n-half, the two chains
            interleaved per m-chunk in the two pv psum slots. Both heads pass
            mc 0-3 together, so the first e-half frees as early as possible
            for the next pair's S chain."""
            eA, eB = e_h
            hA, hB = 2 * p, 2 * p + 1
            psA = pv_psum.tile([HD + 1, NT], F32, tag="pv", name=f"pu{hA}_{nt_}")
            psB = pv_psum.tile([HD + 1, NT], F32, tag="pv", name=f"pu{hB}_{nt_}")
            for mc in range(NMT):
                e_t = eA if mc < 4 else eB
                for ps, j in ((psA, 0), (psB, 1)):
                    nc.tensor.matmul(
                        ps,
                        lhsT=vsb[mc][:, (2 * p + j) * (HD + 1) : (2 * p + j + 1) * (HD + 1)],
                        rhs=e_t[:, ((mc % 4) * 2 + j) * NT : ((mc % 4) * 2 + j + 1) * NT],
                        start=(mc == 0),
                        stop=(mc == NMT - 1),
                    )
            for h, ps in ((hA, psA), (hB, psB)):
                u_t = u_pool.tile([HD + 1, NT], F32, tag="u", name=f"u{h}_{nt_}")
                nc.vector.tensor_copy(u_t, ps)
                pair_units.setdefault((p, nt_), []).append((h, u_t))
                if len(pair_units[(p, nt_)]) == 2:
                    emit_denorm(p, nt_, pair_units.pop((p, nt_)))

        # ---------- output projection + bias ----------
        # dt halves paired on the stationary aot chunk, single [128, 1024]
        # psum per n-tile; alternate between the pj and s psum pools (the s
        # pool is free by the tail) to keep the tail double-buffered.
        wp_tiles = []
        bias_holder = []

        def load_wp():
            bias_sb = one_pool.tile([P, C], F32, tag="bias", name="bias_sb")
            nc.sync.dma_start(bias_sb, bias)
            bias_holder.append(bias_sb)
            for ck in range(NCH):
                wp_t = w_pool.tile([P, N], BF16, tag="wst", name=f"wp{ck}")
                nc.sync.dma_start(wp_t, wpT[ck * P : (ck + 1) * P, :])
                wp_tiles.append(wp_t)

        def emit_outproj():
            bias_sb = bias_holder[0]
            for nt2 in range(NMT):
                pool = pj_psum if nt2 % 2 == 0 else s_psum
                ps = pool.tile(
                    [P, 2 * NT], F32, tag="pj" if nt2 % 2 == 0 else "s",
                    name=f"pso{nt2}",
                )
                for ck in range(NCH):
                    for dt in range(NNT):
                        nc.tensor.matmul(
                            ps[:, dt * NT : (dt + 1) * NT],
                            lhsT=aot[ck][:, nt2 * P : (nt2 + 1) * P],
                            rhs=wp_tiles[ck][:, dt * NT : (dt + 1) * NT],
                            start=(ck == 0),
                            stop=(ck == NCH - 1),
                        )
                o_sb = stage_pool.tile([P, 2 * NT], F32, tag="stage", name=f"o{nt2}")
                nc.vector.tensor_add(o_sb, ps, bias_sb)
                nc.sync.dma_start(out[nt2 * P : (nt2 + 1) * P, :], o_sb)

        # ---------- pipeline ----------
        # pj-psum chain order: qk0, qk1, v(dvt0), qk2, qk3, v(dvt1),
        # qk4..qk7, out-proj. The dvt0 half completes early enough that
        # PV(pairs 0-3) releases e-pool slots before S(pair 2) needs them,
        # and qk(p) always lands ~2 pairs ahead of the ACT pace.
        qks = {0: emit_qkproj(0), 1: emit_qkproj(1)}
        load_wv()
        emit_vproj_half(0)
        qks[2] = emit_qkproj(2)
        qks[3] = emit_qkproj(3)
        emit_vproj_half(1)
        for p in range(NPAIR):
            qp, kp = qks.pop(p)
            e0 = emit_s_exp_nt(p, 0, qp, kp)
            emit_pv_nt(p, 0, e0)
            e1 = emit_s_exp_nt(p, 1, qp, kp)
            emit_pv_nt(p, 1, e1)
            if p >= 2 and p + 2 < NPAIR:
                qks[p + 2] = emit_qkproj(p + 2)
        emit_outproj()

    nc.compile()
    _optimize_pe_stream(nc)
    return nc


def make_in_maps(x, w_qkv, w_proj, b_proj):
    import ml_dtypes

    bf16 = ml_dtypes.bfloat16
    wqkT = np.ascontiguousarray(w_qkv[: 2 * C].T.astype(bf16))
    wvT = np.ascontiguousarray(w_qkv[2 * C :].T.astype(bf16))
    wpT = np.ascontiguousarray(w_proj.T.astype(bf16))
    bias_bc = np.ascontiguousarray(
        np.broadcast_to(b_proj, (P, C)).astype(np.float32)
    )
    ones = np.ones((P, H), dtype=bf16)
    in_maps = []
    for b in range(B):
        in_maps.append(
            {
                "xT": np.ascontiguousarray(x[b].T.astype(bf16)),
                "wqkT": wqkT,
                "wvT": wvT,
                "wpT": wpT,
                "bias_bc": bias_bc,
                "ones_col": ones,
            }
        )
    return in_maps


_CACHED_NC = None


def kernel(x, w_qkv, w_proj, b_proj):
    global _CACHED_NC
    x = np.asarray(x, dtype=np.float32)
    w_qkv = np.asarray(w_qkv, dtype=np.float32)
    w_proj = np.asarray(w_proj, dtype=np.float32)
    b_proj = np.asarray(b_proj, dtype=np.float32)
    if _CACHED_NC is None:
        _CACHED_NC = build_module()
    nc = _CACHED_NC
    in_maps = make_in_maps(x, w_qkv, w_proj, b_proj)
    res = bass_utils.run_bass_kernel_spmd(nc, in_maps, core_ids=list(range(B)))
    return np.stack([res.results[b]["out"] for b in range(B)], axis=0)


if __name__ == "__main__":
    nc = build_module()
    ninst = sum(len(b.instructions) for b in nc.m.functions[0].blocks)
    print("module built ok;", ninst, "instructions")


# revision 52
# speedup vs baseline: 1.0067x; 1.0067x over previous
"""Multi-head attention (B=8, N=1024, C=1024, H=16) on 8 Trainium2 NeuronCores.

Sharding: pure data-parallel — one batch element per core, weights replicated,
no collectives.

v2 design (vs baseline): bf16 matmul operands everywhere (PSUM accumulation
stays fp32), no DRAM bounce for qk (everything lives in SBUF), per-head-pair
software pipeline so the PE and ACT engines run concurrently, 1024-wide exp
activations (psum tiles spanning 2 banks) to amortize ACT fixed overhead, and
row-packed S matmuls (K=64 head A on array rows 0-63, head B on rows 64-127,
issued back-to-back so they execute concurrently).

Per-core algorithm:
  v-proj:    v[m, dv] natural layout, interleaved [m, 16*(64+1)] with a ones
             column per head (PV then emits softmax denominators for free).
  qk-proj:   per pair p: qp[c(2 heads), n], kp[c, m] bf16 tiles in SBUF.
  attention: per (pair, nt half):
               4 groups: S^T chunks for heads A,B into [128,1024] psum pairs,
               exp (ACT, 1024-wide, scale=1/8) -> eA/eB bf16 [128, 4096]
               PV: U_aug[65, nt] = v_aug.T @ expS accumulated over 8 m-chunks
             U -> SBUF (fp32, frees psum), denominators row 64 -> reciprocal
             -> DRAM bounce -> partition-broadcast -> normalize on GpSimd into
             attn_outT[c, n] bf16.
  out-proj:  out[n, d] = attn_outT.T @ wpT + bias, fp32 out.
"""

import sys

if "/opt/trn_rl_repo" not in sys.path:
    sys.path.insert(0, "/opt/trn_rl_repo")

from contextlib import ExitStack

import numpy as np

import concourse.bass as bass
import concourse.mybir as mybir
from concourse import bacc
import concourse.tile as tile
from concourse import bass_utils

B, N, C, H = 8, 1024, 1024, 16
HD = C // H          # 64
SCALE = HD ** -0.5   # 0.125
P = 128              # SBUF partitions
NT = 512             # moving-dim tile (fp32 PSUM bank limit)
NCH = C // P         # 8 contraction chunks over channels
NMT = N // P         # 8 token tiles of 128
NNT = N // NT        # 2 token tiles of 512
NPAIR = H // 2       # 8 head pairs
F32 = mybir.dt.float32
BF16 = mybir.dt.bfloat16
EXP = mybir.ActivationFunctionType.Exp


def _wait_key(w):
    return (w.sync_type, w.id, w.wait_mode, w.wait_value)


def _weights_sig(ldw):
    a = ldw.ins[0]
    return (a.memref, a.offset, tuple(tuple(x) for x in a.ap), str(a.dtype))


def _optimize_pe_stream(nc):
    """Post-compile peephole pass over the PE instruction stream.

    Operates on the PE-only subsequence (other engines' instructions are
    interleaved in the block list but the PE sequencer only sees its own
    stream; cross-engine ordering is carried entirely by semaphores).

    Rule 1 (dedupe): a wait-free LDWEIGHTS reloading the weights already in
      the array (and already consumed by a matmul) is deleted. Wait-carrying
      LDWs are kept: waits only function on LDWEIGHTS (the PE hw-decoder
      ignores waits on MATMUL), and the first load of a compiler
      [LDW, LDW, MM, MM] prefetch pair may run before the DMA-complete wait.
    Rule 2 (hoist): [LDW_A, MM_A@(0,0) K=64, LDW_B, MM_B@(64,0) K=64] ->
      [LDW_A, LDW_B, MM_A, MM_B] so the two matmuls execute concurrently on
      disjoint row groups; only when LDW_B's waits are implied by LDW_A's
      (same semaphore, same-or-lower threshold), so the earlier wait position
      cannot deadlock.
    """
    import concourse.mybir as mybir
    from collections import deque

    n_dedupe = n_hoist = 0
    for f in nc.m.functions:
        for blk in f.blocks:
            insts = blk.instructions
            pe = [a for a in insts if getattr(a, "engine", None) == mybir.EngineType.PE]
            out = []
            i = 0
            n = len(pe)
            cur_sig = None
            cur_consumed = False
            while i < n:
                a = pe[i]
                if isinstance(a, mybir.InstLdweights):
                    sig = _weights_sig(a)
                    if (
                        sig == cur_sig
                        and cur_consumed
                        and (
                            a.sync_info is None
                            or not (a.sync_info.on_wait or a.sync_info.on_update)
                        )
                        and i + 1 < n
                        and isinstance(pe[i + 1], mybir.InstMatmult)
                    ):
                        out.append(pe[i + 1])
                        i += 2
                        n_dedupe += 1
                        cur_consumed = True
                        continue
                    if (
                        i + 3 < n
                        and isinstance(pe[i + 1], mybir.InstMatmult)
                        and isinstance(pe[i + 2], mybir.InstLdweights)
                        and isinstance(pe[i + 3], mybir.InstMatmult)
                    ):
                        ldw1, mm1, ldw2, mm2 = pe[i : i + 4]
                        tp1 = mm1.tile_position
                        tp2 = mm2.tile_position
                        if (
                            tp1 is not None
                            and tp2 is not None
                            and tuple(tp1) == (0, 0)
                            and tuple(tp2) == (64, 0)
                            and ldw1.ins[0].ap[0][1] == 64
                            and ldw2.ins[0].ap[0][1] == 64
                        ):
                            w1 = [
                                _wait_key(w)
                                for w in (
                                    ldw1.sync_info.on_wait if ldw1.sync_info else []
                                )
                            ]
                            w2 = [
                                _wait_key(w)
                                for w in (
                                    ldw2.sync_info.on_wait if ldw2.sync_info else []
                                )
                            ]
                            implied = all(
                                any(
                                    k[0] == kk[0]
                                    and k[1] == kk[1]
                                    and k[2] == kk[2]
                                    and k[3] <= kk[3]
                                    for kk in w1
                                )
                                for k in w2
                            )
                            if implied and not (
                                ldw2.sync_info and ldw2.sync_info.on_update
                            ):
                                out.extend([ldw1, ldw2, mm1, mm2])
                                cur_sig = _weights_sig(ldw2)
                                cur_consumed = True
                                i += 4
                                n_hoist += 1
                                continue
                    cur_sig = sig
                    cur_consumed = False
                elif isinstance(a, mybir.InstMatmult):
                    cur_consumed = True
                else:
                    cur_sig = None  # unknown PE instruction: be conservative
                    cur_consumed = False
                out.append(a)
                i += 1
            # weave the transformed PE stream back into the block, keeping
            # non-PE instructions in place; trailing PE slots left over from
            # deletions are simply skipped.
            pe_q = deque(out)
            new_insts = []
            for a in insts:
                if getattr(a, "engine", None) == mybir.EngineType.PE:
                    if pe_q:
                        new_insts.append(pe_q.popleft())
                else:
                    new_insts.append(a)
            assert not pe_q, "transformed PE stream longer than original slots"
            blk.instructions = new_insts
    print(f"_optimize_pe_stream: {n_dedupe} LDW deduped, {n_hoist} LDW hoisted")


def build_module():
    nc = bacc.Bacc("TRN2", target_bir_lowering=False, debug=False, num_devices=B)

    xT = nc.dram_tensor("xT", [C, N], BF16, kind="ExternalInput").ap()
    wqkT = nc.dram_tensor("wqkT", [C, 2 * C], BF16, kind="ExternalInput").ap()
    wvT = nc.dram_tensor("wvT", [C, C], BF16, kind="ExternalInput").ap()
    wpT = nc.dram_tensor("wpT", [C, C], BF16, kind="ExternalInput").ap()
    bias = nc.dram_tensor("bias_bc", [P, C], F32, kind="ExternalInput").ap()
    ones_col = nc.dram_tensor("ones_col", [P, H], BF16, kind="ExternalInput").ap()
    out = nc.dram_tensor("out", [N, C], F32, kind="ExternalOutput").ap()

    with tile.TileContext(nc) as tc, ExitStack() as ctx:
        dram = ctx.enter_context(tc.tile_pool(name="dram", bufs=1, space="DRAM"))
        rden_d = dram.tile([H * NNT, NT], F32, tag="rden_d", name="rden_d")

        xt_pool = ctx.enter_context(tc.tile_pool(name="xt", bufs=8))
        qk_pool = ctx.enter_context(tc.tile_pool(name="qk", bufs=8))
        v_pool = ctx.enter_context(tc.tile_pool(name="v", bufs=8))
        e_pool = ctx.enter_context(tc.tile_pool(name="e", bufs=8))
        u_pool = ctx.enter_context(tc.tile_pool(name="u", bufs=6))
        aot_pool = ctx.enter_context(tc.tile_pool(name="aot", bufs=1))
        w_pool = ctx.enter_context(tc.tile_pool(name="wst", bufs=9))
        wqk_pool = ctx.enter_context(tc.tile_pool(name="wqk", bufs=24))
        den_pool = ctx.enter_context(tc.tile_pool(name="den", bufs=4))
        rbc_pool = ctx.enter_context(tc.tile_pool(name="rbc", bufs=6))
        one_pool = ctx.enter_context(tc.tile_pool(name="one", bufs=1))
        stage_pool = ctx.enter_context(tc.tile_pool(name="stage", bufs=2))
        s_psum = ctx.enter_context(tc.tile_pool(name="s_ps", bufs=2, space="PSUM"))
        pv_psum = ctx.enter_context(tc.tile_pool(name="pv_ps", bufs=2, space="PSUM"))
        pj_psum = ctx.enter_context(tc.tile_pool(name="pj_ps", bufs=1, space="PSUM"))

        # ---------- input loads ----------
        xts = []
        for t in range(NCH):
            xt_t = xt_pool.tile([P, N], BF16, tag="xt", name=f"xt{t}")
            nc.sync.dma_start(xt_t, xT[t * P : (t + 1) * P, :])
            xts.append(xt_t)
        vsb = []
        for mt in range(NMT):
            v_t = v_pool.tile([P, H * (HD + 1)], BF16, tag="v", name=f"v{mt}")
            nc.sync.dma_start(
                v_t.rearrange("p (h w) -> p h w", w=HD + 1)[:, :, HD : HD + 1], ones_col
            )
            vsb.append(v_t)
        aot = [
            aot_pool.tile([P, N], BF16, tag=f"aot{t}", name=f"aot{t}")
            for t in range(NCH)
        ]

        # ---------- v projection (natural layout + ones cols) ----------
        # Runs on the pv psum banks (idle until the first PV at ~50us) so the
        # projection chain proceeds concurrently with the qk-proj chain on the
        # pj slot; per-dvt [128,512] blocks ping-pong the two pv banks. PV(p)
        # only needs vsb[mc], so it chases these blocks per-mt.
        def emit_vproj():
            wv_tiles = []
            for ck in range(NCH):
                wv_t = w_pool.tile([P, N], BF16, tag="wst", name=f"wv{ck}")
                nc.sync.dma_start(wv_t, wvT[ck * P : (ck + 1) * P, :])
                wv_tiles.append(wv_t)
            for dvt in range(NNT):
                for mt in range(NMT):
                    ps = pv_psum.tile([P, NT], F32, tag="pv", name=f"psv{dvt}_{mt}")
                    for ck in range(NCH):
                        nc.tensor.matmul(
                            ps,
                            lhsT=xts[ck][:, mt * P : (mt + 1) * P],
                            rhs=wv_tiles[ck][:, dvt * NT : (dvt + 1) * NT],
                            start=(ck == 0),
                            stop=(ck == NCH - 1),
                        )
                    dst = vsb[mt].rearrange("p (h w) -> p h w", w=HD + 1)[
                        :, dvt * 8 : (dvt + 1) * 8, 0:HD
                    ]
                    nc.vector.tensor_copy(dst, ps.rearrange("p (h w) -> p h w", w=HD))

        # ---------- qk projection for one head pair, SBUF-resident ----------
        def emit_qkproj(p):
            qp = qk_pool.tile([P, N], BF16, tag="qk", name=f"qp{p}")
            kp = qk_pool.tile([P, N], BF16, tag="qk", name=f"kp{p}")
            for which, col0, dstt in ((0, p * P, qp), (1, C + p * P, kp)):
                wts = []
                for ck in range(NCH):
                    w_t = wqk_pool.tile(
                        [P, P], BF16, tag="wqk", name=f"w{which}_{p}_{ck}"
                    )
                    nc.sync.dma_start(w_t, wqkT[ck * P : (ck + 1) * P, col0 : col0 + P])
                    wts.append(w_t)
                ps = pj_psum.tile([P, 2 * NT], F32, tag="pj", name=f"psqk{which}_{p}")
                for ck in range(NCH):
                    for nt_ in range(NNT):
                        nc.tensor.matmul(
                            ps[:, nt_ * NT : (nt_ + 1) * NT],
                            lhsT=wts[ck],
                            rhs=xts[ck][:, nt_ * NT : (nt_ + 1) * NT],
                            start=(ck == 0),
                            stop=(ck == NCH - 1),
                        )
                nc.vector.tensor_copy(dstt, ps)
            return qp, kp

        # ---------- attention ----------
        pair_units = {}

        def emit_denorm(p, nt_, punits):
            # punits: list of (h, nt, u_t). Pairs 0-6 batch all four units
            # (one den DMA group mid-stream); the last pair runs per n-half
            # so out-proj blocks 0-3 (which only read the nt0 half of aot)
            # unblock as early as possible.
            k = len(punits)
            den_g = den_pool.tile([k, NT], F32, tag="den", name=f"den{p}_{nt_}")
            for i, (h, unt, u_t) in enumerate(punits):
                nc.sync.dma_start(den_g[i : i + 1, :], u_t[HD : HD + 1, :])
            rden = den_pool.tile([k, NT], F32, tag="rden", name=f"rden{p}_{nt_}")
            nc.vector.reciprocal_approx_fast(out=rden, in_=den_g)
            row0 = p * 4 + (nt_ or 0) * 2 if k == 2 else p * 4
            nc.sync.dma_start(rden_d[row0 : row0 + k, :], rden)
            for i, (h, unt, u_t) in enumerate(punits):
                rbc = rbc_pool.tile([HD, NT], F32, tag="rbc", name=f"rbc{h}_{nt_}")
                src_ = rden_d[row0 + i : row0 + i + 1, :]
                bsrc = bass.AP(
                    tensor=src_.tensor,
                    offset=src_.offset,
                    ap=[[0, HD], list(src_.ap[-1])],
                )
                nc.sync.dma_start(out=rbc, in_=bsrc)
                ct, prow = h // 2, (h % 2) * HD
                # split normalize across GpSimd and Vector so the two heads'
                # muls run on different engines (matters for the last pair's
                # tail, which gates out-proj).
                eng = nc.gpsimd if h % 2 == 0 else nc.vector
                eng.tensor_mul(
                    aot[ct][prow : prow + HD, unt * NT : (unt + 1) * NT],
                    u_t[0:HD, :],
                    rbc,
                )

        def emit_s_exp_nt(p, nt_, qp, kp):
            """S^T + exp for one (pair, n-half). Heads A and B share one
            [128, 1024] psum tile per m-chunk (A in the low bank, B in the
            high bank) so both matmuls become ready together; the post-compile
            pass hoists B's LDWEIGHTS above A's matmul, making the two K=64
            matmuls (array rows 0-63 / 64-127) run concurrently.
            Returns two e tiles [128, 4096] (mc 0-3 and mc 4-7), each laid
            out as [A_mc|B_mc|...]; the split lets PV release the first half
            mid-chain so the next pair's S can reuse the slots earlier."""
            eA = e_pool.tile([P, NMT * NT], BF16, tag="e", name=f"e{p}_{nt_}a")
            eB = e_pool.tile([P, NMT * NT], BF16, tag="e", name=f"e{p}_{nt_}b")
            for mc in range(NMT):
                e_t = eA if mc < 4 else eB
                s_t = s_psum.tile([P, 2 * NT], F32, tag="s", name=f"s{p}_{nt_}_{mc}")
                # high priority: the S pair feeds ACT (the attention-phase
                # pacer) and must pop back-to-back so the post-compile hoist
                # can make the two K=64 row-tiles run concurrently.
                with tc.high_priority():
                    nc.tensor.matmul(
                        s_t[:, 0:NT],
                        lhsT=kp[0:HD, mc * P : (mc + 1) * P],
                        rhs=qp[0:HD, nt_ * NT : (nt_ + 1) * NT],
                        start=True,
                        stop=True,
                    )
                    nc.tensor.matmul(
                        s_t[:, NT : 2 * NT],
                        lhsT=kp[HD:P, mc * P : (mc + 1) * P],
                        rhs=qp[HD:P, nt_ * NT : (nt_ + 1) * NT],
                        start=True,
                        stop=True,
                    )
                nc.scalar.activation(
                    e_t[:, (mc % 4) * 2 * NT : ((mc % 4) + 1) * 2 * NT],
                    s_t,
                    EXP,
                    scale=SCALE,
                )
            return eA, eB

        def emit_pv_nt(p, nt_, e_h):
            """PV for BOTH heads of the pair over one n-half, the two chains
            interleaved per m-chunk in the two pv psum slots. Both heads pass
            mc 0-3 together, so the first e-half frees as early as possible
            for the next pair's S chain."""
            eA, eB = e_h
            hA, hB = 2 * p, 2 * p + 1
            psA = pv_psum.tile([HD + 1, NT], F32, tag="pv", name=f"pu{hA}_{nt_}")
            psB = pv_psum.tile([HD + 1, NT], F32, tag="pv", name=f"pu{hB}_{nt_}")
            for mc in range(NMT):
                e_t = eA if mc < 4 else eB
                for ps, j in ((psA, 0), (psB, 1)):
                    nc.tensor.matmul(
                        ps,
                        lhsT=vsb[mc][:, (2 * p + j) * (HD + 1) : (2 * p + j + 1) * (HD + 1)],
                        rhs=e_t[:, ((mc % 4) * 2 + j) * NT : ((mc % 4) * 2 + j + 1) * NT],
                        start=(mc == 0),
                        stop=(mc == NMT - 1),
                    )
            for h, ps in ((hA, psA), (hB, psB)):
                u_t = u_pool.tile([HD + 1, NT], F32, tag="u", name=f"u{h}_{nt_}")
                nc.vector.tensor_copy(u_t, ps)
                pair_units.setdefault((p, nt_), []).append((h, nt_, u_t))
                if p == NPAIR - 1 and len(pair_units[(p, nt_)]) == 2:
                    emit_denorm(p, nt_, pair_units.pop((p, nt_)))
                elif p < NPAIR - 1:
                    both = pair_units.get((p, 0), []) + pair_units.get((p, 1), [])
                    if len(both) == 4:
                        pair_units.pop((p, 0))
                        pair_units.pop((p, 1))
                        emit_denorm(p, None, both)

        # ---------- output projection + bias ----------
        # dt halves paired on the stationary aot chunk, single [128, 1024]
        # psum per n-tile; alternate between the pj and s psum pools (the s
        # pool is free by the tail) to keep the tail double-buffered.
        wp_tiles = []
        bias_holder = []

        def load_wp():
            bias_sb = one_pool.tile([P, C], F32, tag="bias", name="bias_sb")
            nc.sync.dma_start(bias_sb, bias)
            bias_holder.append(bias_sb)
            for ck in range(NCH):
                wp_t = w_pool.tile([P, N], BF16, tag="wst", name=f"wp{ck}")
                nc.sync.dma_start(wp_t, wpT[ck * P : (ck + 1) * P, :])
                wp_tiles.append(wp_t)

        def emit_outproj():
            bias_sb = bias_holder[0]
            for nt2 in range(NMT):
                pool = pj_psum if nt2 % 2 == 0 else s_psum
                ps = pool.tile(
                    [P, 2 * NT], F32, tag="pj" if nt2 % 2 == 0 else "s",
                    name=f"pso{nt2}",
                )
                for ck in range(NCH):
                    for dt in range(NNT):
                        nc.tensor.matmul(
                            ps[:, dt * NT : (dt + 1) * NT],
                            lhsT=aot[ck][:, nt2 * P : (nt2 + 1) * P],
                            rhs=wp_tiles[ck][:, dt * NT : (dt + 1) * NT],
                            start=(ck == 0),
                            stop=(ck == NCH - 1),
                        )
                o_sb = stage_pool.tile([P, 2 * NT], F32, tag="stage", name=f"o{nt2}")
                nc.vector.tensor_add(o_sb, ps, bias_sb)
                nc.sync.dma_start(out[nt2 * P : (nt2 + 1) * P, :], o_sb)

        # ---------- pipeline ----------
        # pj-psum chain order: qk0, qk1, v(dvt0), qk2, qk3, v(dvt1),
        # qk4..qk7, out-proj. The dvt0 half completes early enough that
        # PV(pairs 0-3) releases e-pool slots before S(pair 2) needs them,
        # and qk(p) always lands ~2 pairs ahead of the ACT pace.
        qks = {0: emit_qkproj(0), 1: emit_qkproj(1)}
        load_wv()
        emit_vproj_half(0)
        qks[2] = emit_qkproj(2)
        qks[3] = emit_qkproj(3)
        emit_vproj_half(1)
        for p in range(NPAIR):
            qp, kp = qks.pop(p)
            e0 = emit_s_exp_nt(p, 0, qp, kp)
            emit_pv_nt(p, 0, e0)
            e1 = emit_s_exp_nt(p, 1, qp, kp)
            emit_pv_nt(p, 1, e1)
            if p >= 2 and p + 2 < NPAIR:
                qks[p + 2] = emit_qkproj(p + 2)
        emit_outproj()

    nc.compile()
    _optimize_pe_stream(nc)
    return nc


def make_in_maps(x, w_qkv, w_proj, b_proj):
    import ml_dtypes

    bf16 = ml_dtypes.bfloat16
    wqkT = np.ascontiguousarray(w_qkv[: 2 * C].T.astype(bf16))
    wvT = np.ascontiguousarray(w_qkv[2 * C :].T.astype(bf16))
    wpT = np.ascontiguousarray(w_proj.T.astype(bf16))
    bias_bc = np.ascontiguousarray(
        np.broadcast_to(b_proj, (P, C)).astype(np.float32)
    )
    ones = np.ones((P, H), dtype=bf16)
    in_maps = []
    for b in range(B):
        in_maps.append(
            {
                "xT": np.ascontiguousarray(x[b].T.astype(bf16)),
                "wqkT": wqkT,
                "wvT": wvT,
                "wpT": wpT,
                "bias_bc": bias_bc,
                "ones_col": ones,
            }
        )
    return in_maps


_CACHED_NC = None


def kernel(x, w_qkv, w_proj, b_proj):
    global _CACHED_NC
    x = np.asarray(x, dtype=np.float32)
    w_qkv = np.asarray(w_qkv, dtype=np.float32)
    w_proj = np.asarray(w_proj, dtype=np.float32)
    b_proj = np.asarray(b_proj, dtype=np.float32)
    if _CACHED_NC is None:
        _CACHED_NC = build_module()
    nc = _CACHED_NC
    in_maps = make_in_maps(x, w_qkv, w_proj, b_proj)
    res = bass_utils.run_bass_kernel_spmd(nc, in_maps, core_ids=list(range(B)))
    return np.stack([res.results[b]["out"] for b in range(B)], axis=0)


if __name__ == "__main__":
    nc = build_module()
    ninst = sum(len(b.instructions) for b in nc.m.functions[0].blocks)
    print("module built ok;", ninst, "instructions")
